# revision 37
# baseline (speedup 1.0000x reference)
"""Hierarchical-softmax loss kernel for Trainium2 (8 NeuronCores).

Strategy
--------
Data-parallel over the n_ex dimension. Examples are globally sorted by
path length (descending) and dealt round-robin to the 8 cores so every
core sees a near-identical length profile. Within a core, examples are
tiled into 8 partition-tiles of 128; each tile's step loop runs only to
that tile's max length (sum ~114 of the dense 192 steps).

W is cast to bf16 on the host, halving the dominant gather traffic
(~29 MB/core HBM reads, ~81 us at the measured ~360 GB/s). Gathers are
batched 8 steps (1024 rows) per indirect DMA: SWDGE descriptor
generation costs ~1 us fixed per instruction but only 0.34 ns per
descriptor, so batching cuts the GpSimd engine load ~7x vs per-step
gathers (134 us -> ~30 us).

Per chunk, one batched DVE multiply (bf16 2x mode, ~440-520 ns/step)
produces all products; the per-step dot reductions are then split
11/16 to ScalarE copy-accumulate (~1.28 us/step) and 5/16 to a single
grouped DVE tensor_reduce (~0.96 us/step) so both engines finish
together (~97 us each, the kernel's wall). Fused mult+reduce ops
(tensor_tensor_reduce, custom-DVE affine_mul_reduce) do NOT compile on
the pinned walrus ("ISA wrong length"), and GpSimd tensor compute
inflates DVE/ACT times 25-30% via SBUF port contention, so neither is
used. First chunks are tapered (2/3/5 steps) and x tiles load lazily
to shorten the pipeline ramp; the last chunks lean on DVE because
ScalarE is the post-DMA straggler. The final softplus tail runs once
over all dots; the scalar sum over 8x128 partials happens on host.
"""

import os
import sys

import numpy as np

for _p in ("/opt/trn_rl_repo", "/root/.axon_site/_ro/trn_rl_repo"):
    if os.path.isdir(_p) and _p not in sys.path:
        sys.path.append(_p)

V = 50257
N_DEC = V - 1
D = 1024
N_EX = 8192
MAX_LEN = 24
N_CORES = 8
P = 128
N_TILES = N_EX // (N_CORES * P)  # 8 example-tiles of 128 per core
MASK_BIAS = -30.0                # softplus(-30) ~ 9e-14 == masked-out step

CHUNK = 8   # steps per gather batch
NA_NUM, NA_DEN = 11, 12  # fraction of steps reduced on ScalarE (rest on DVE)
# GpSimd compute is disabled: Q7 SBUF traffic contends with DVE 2x-port mode
# and inflates DVE/ACT instruction times ~25% (measured).
NP_NUM, NP_DEN = 0, 16   # fraction of steps whose multiply runs on GpSimd
PF = 3      # chunks of gather prefetch ahead of compute
FUSED_FP8 = False   # (fused DVE ops don't compile on the pinned walrus)
W8_SCALE = 32.0     # host-side W scale so e4m3 stays in normal range

_prog_cache: dict = {}


def _plan(lmax):
    """Chunk schedule shared by host prep and program builder.

    Returns [(k, c0, c, na, nf)]: tile k, step range [c0, c0+c), na steps
    ScalarE-reduced (first na of the chunk), nf = c - na fused on DVE.
    """
    chunks = []
    acc = 0  # Bresenham accumulator so the ScalarE fraction holds globally
    for k, lm in enumerate(lmax):
        lm = int(lm)
        if k == 0 and lm > CHUNK:
            # taper the first chunks so the pipeline fills quickly
            sizes = [2, 3, 5]
            while sum(sizes) + CHUNK <= lm:
                sizes.append(CHUNK)
            rest = lm - sum(sizes)
            if rest > 0:
                sizes.append(rest)
        else:
            sizes = [CHUNK] * (lm // CHUNK)
            if lm % CHUNK:
                sizes.append(lm % CHUNK)
        c0 = 0
        for c in sizes:
            acc += c * NA_NUM
            na = acc // NA_DEN
            acc -= na * NA_DEN
            chunks.append([k, c0, c, na, c - na])
            c0 += c
    # tail chunks lean on DVE: ScalarE accums are the post-DMA straggler
    for ch in chunks[-2:]:
        ch[3] = ch[2] // 3
        ch[4] = ch[2] - ch[3]
    return [tuple(ch) for ch in chunks]


def _patch_tail_drain(tile, mybir, bass_rust):
    """The pinned walrus encodes only a limited number of sync-waits per CTRL
    instruction, but Tile's kernel-tail Drain carries one wait per active
    processor lane. Spread the extra waits over single-wait NOPs."""
    if getattr(tile.TileContext._drain_and_barrier, "_split_waits", False):
        return

    def _drain_and_barrier(self, tick_clock, wait_clock):
        nc = self.nc
        drain_inst = nc.sync.drain()
        wait_clock.add_sem_waits(
            drain_inst.ins, bass_rust.ScopedClock({None: tick_clock.global_clock})
        )
        si = drain_inst.ins.sync_info
        waits = list(si.on_wait or [])
        if len(waits) > 1:
            si.on_wait = waits[:1]
            for w in waits[1:]:
                nop = nc.sync.nop(nofuse=True)
                nop.ins.sync_info = mybir.SyncInfo(on_wait=[w], on_update=[])
        nc.all_engine_barrier()
        popped = nc._tile_sem_poison_stack.pop()
        assert popped is self._sem_poison
        nc.clear_and_free_semaphores(list(self.sems.allocated().values()))
        nc.all_engine_barrier()

    _drain_and_barrier._split_waits = True
    tile.TileContext._drain_and_barrier = _drain_and_barrier


def _split_multiwait_instructions(nc, mybir, maxw=1):
    """Hoist extra sem-waits from any instruction onto single-wait NOPs placed
    immediately before it on the same engine (same aggregate wait semantics)."""
    f = nc.m.functions[0]
    tail = nc.cur_bb.bb
    blocks = list(f.blocks)
    if not any(b.name == tail.name for b in blocks):
        blocks.append(tail)
    for blk in blocks:
        snapshot = list(blk.instructions)
        heavy = [
            i for i in snapshot
            if i.sync_info and i.sync_info.on_wait and len(i.sync_info.on_wait) > maxw
        ]
        if not heavy:
            continue
        pre_len = len(tail.instructions)
        n_created = 0
        new_list = []
        for inst in snapshot:
            si = inst.sync_info
            if si and si.on_wait and len(si.on_wait) > maxw:
                waits = list(si.on_wait)
                extra, keep = waits[:-maxw], waits[-maxw:]
                si.on_wait = keep
                for w in extra:
                    nop = nc.engines[inst.engine].nop(nofuse=True)
                    nop.ins.sync_info = mybir.SyncInfo(on_wait=[w], on_update=[])
                    new_list.append(nop.ins)
                    n_created += 1
            new_list.append(inst)
        # builder appended the fresh NOPs to the tail block; strip them there
        t = list(tail.instructions)
        assert len(t) == pre_len + n_created
        if blk.name == tail.name:
            blk.instructions = new_list
        else:
            tail.instructions = t[:pre_len]
            blk.instructions = new_list


def _build_program(lmax: tuple):
    from concourse import bass, mybir
    import concourse.tile as tile
    import bass_rust

    _patch_tail_drain(tile, mybir, bass_rust)
    chunks = _plan(lmax)
    ltot = int(sum(lmax))

    nc = bass.Bass("TRN2", target_bir_lowering=False,
                   dynamic_dma_scratch_size=2 ** 16)
    f32 = mybir.dt.float32
    bf16 = mybir.dt.bfloat16

    xs = nc.declare_dram_parameter("xs", [N_TILES * P, D], bf16, isOutput=False)
    W16 = nc.declare_dram_parameter("W16", [N_DEC, D], bf16, isOutput=False)
    if FUSED_FP8:
        W8 = nc.declare_dram_parameter("W8", [N_DEC, D], mybir.dt.float8e4,
                                       isOutput=False)
    gidx = nc.declare_dram_parameter("gidx", [P, ltot], mybir.dt.int32, isOutput=False)
    nsc = nc.declare_dram_parameter("nsc", [P, N_TILES * MAX_LEN], f32, isOutput=False)
    mbs = nc.declare_dram_parameter("mbs", [P, N_TILES * MAX_LEN], f32, isOutput=False)
    out = nc.declare_dram_parameter("out", [P, 1], f32, isOutput=True)

    it0 = np.concatenate([[0], np.cumsum(lmax)]).astype(int)  # gidx col base per tile

    with tile.TileContext(nc) as tc:
        with (
            tc.tile_pool(name="xpool", bufs=1) as xpool,
            tc.tile_pool(name="gpool", bufs=PF + 1) as gpool,
            tc.tile_pool(name="meta", bufs=1) as meta,
            tc.tile_pool(name="pspool", bufs=3) as pspool,
            tc.tile_pool(name="dpool", bufs=2) as dpool,
            tc.tile_pool(name="outp", bufs=1) as outp,
        ):
            gidx_t = meta.tile([P, ltot], mybir.dt.int32, tag="gidx")
            nsc_t = meta.tile([P, N_TILES * MAX_LEN], f32, tag="nsc")
            mbs_t = meta.tile([P, N_TILES * MAX_LEN], f32, tag="mbs")
            nc.sync.dma_start(out=gidx_t[:], in_=gidx[:, :])

            parts = outp.tile([P, 1], f32, tag="parts")
            pa = outp.tile([P, 1], f32, tag="pa")
            pb = outp.tile([P, 1], f32, tag="pb")

            # x tiles load lazily (tile k's load is emitted one tile ahead of
            # first use) so the startup DMA window belongs to the gathers
            xt = []
            for k in range(N_TILES):
                xt.append(xpool.tile([P, D], bf16, tag=f"x{k}", name=f"x{k}"))
            x_loaded = [False] * N_TILES

            def load_x(k):
                if k < N_TILES and not x_loaded[k]:
                    nc.sync.dma_start(out=xt[k][:], in_=xs[k * P : (k + 1) * P, :])
                    x_loaded[k] = True

            load_x(0)
            load_x(1)

            # one dots buffer for all tiles; padded columns stay 0 via memset
            dots = dpool.tile([P, N_TILES * MAX_LEN], f32, tag="dots")
            nc.vector.memset(dots[:], 0.0)

            g_tiles = {}

            def emit_gather(i):
                k, c0, c, na, nd = chunks[i]
                g = gpool.tile([P, CHUNK * D], bf16, tag="g")
                nc.gpsimd.indirect_dma_start(
                    out=g[:, : c * D],
                    out_offset=None,
                    in_=W16[:, :],
                    in_offset=bass.IndirectOffsetOnAxis(
                        ap=gidx_t[:, it0[k] + c0 : it0[k] + c0 + c], axis=0
                    ),
                )
                g_tiles[i] = g

            for i in range(min(PF + 1, len(chunks))):
                emit_gather(i)

            for i, (k, c0, c, na, nd) in enumerate(chunks):
                g = g_tiles.pop(i)
                dbase = k * MAX_LEN + c0
                load_x(k + 1)
                if i == len(chunks) - 6:
                    # gather stream is winding down: fetch the tail tables
                    nc.sync.dma_start(out=nsc_t[:], in_=nsc[:, :])
                    nc.sync.dma_start(out=mbs_t[:], in_=mbs[:, :])
                ps = pspool.tile([P, CHUNK * D], bf16, tag="ps", bufs=2)
                x3 = xt[k][:].unsqueeze(1).broadcast_to([P, c, D])
                nc.vector.tensor_tensor(
                    out=ps[:, : c * D].rearrange("p (n d) -> p n d", d=D),
                    in0=x3,
                    in1=g[:, : c * D].rearrange("p (n d) -> p n d", d=D),
                    op=mybir.AluOpType.mult,
                )
                # one 2x half-add pass halves every downstream reduce read
                H = D // 2
                ph = pspool.tile([P, CHUNK * H], bf16, tag="ph")
                ps3 = ps[:, : c * D].rearrange("p (n d) -> p n d", d=D)
                nc.vector.tensor_tensor(
                    out=ph[:, : c * H].rearrange("p (n h) -> p n h", h=H),
                    in0=ps3[:, :, :H],
                    in1=ps3[:, :, H:],
                    op=mybir.AluOpType.add,
                )
                dump = pspool.tile([P, H], bf16, tag="dump")
                for j in range(na):
                    nc.scalar.activation(
                        out=dump[:], in_=ph[:, j * H : (j + 1) * H],
                        func=mybir.ActivationFunctionType.Copy,
                        accum_out=dots[:, dbase + j : dbase + j + 1],
                    )
                if nd > 0:
                    nc.vector.tensor_reduce(
                        out=dots[:, dbase + na : dbase + c],
                        in_=ph[:, na * H : c * H].rearrange(
                            "p (n h) -> p n h", h=H
                        ),
                        axis=mybir.AxisListType.X,
                        op=mybir.AluOpType.add,
                    )
                if i + PF + 1 < len(chunks):
                    emit_gather(i + PF + 1)

            # v = dot * (-code) + mask_bias over all tiles at once
            nc.vector.tensor_tensor(
                out=dots[:], in0=dots[:], in1=nsc_t[:], op=mybir.AluOpType.mult
            )
            nc.vector.tensor_tensor(
                out=dots[:], in0=dots[:], in1=mbs_t[:], op=mybir.AluOpType.add
            )
            # stable softplus(v) = relu(v) + ln(1 + exp(-|v|));
            # abs/exp/ln/relu share one ACT table set (natural_log_exp).
            va = dpool.tile([P, N_TILES * MAX_LEN], f32, tag="va")
            nc.scalar.activation(
                out=va[:], in_=dots[:], func=mybir.ActivationFunctionType.Abs
            )
            ve = dpool.tile([P, N_TILES * MAX_LEN], f32, tag="ve")
            nc.scalar.activation(
                out=ve[:], in_=va[:],
                func=mybir.ActivationFunctionType.Exp, scale=-1.0,
            )
            vl = dpool.tile([P, N_TILES * MAX_LEN], f32, tag="vl")
            nc.scalar.activation(
                out=vl[:], in_=ve[:],
                func=mybir.ActivationFunctionType.Ln, bias=1.0,
                accum_out=pb[:, :],
            )
            vr = dpool.tile([P, N_TILES * MAX_LEN], f32, tag="vr")
            nc.scalar.activation(
                out=vr[:], in_=dots[:],
                func=mybir.ActivationFunctionType.Relu,
                accum_out=pa[:, :],
            )

            nc.vector.tensor_tensor(
                out=parts[:], in0=pa[:], in1=pb[:], op=mybir.AluOpType.add
            )
            nc.sync.dma_start(out=out[:, :], in_=parts[:])

    _split_multiwait_instructions(nc, mybir)
    return nc


def _prepare(x, W, t, paths, codes, lens):
    """Host-side index prep: length-sorted round-robin shard + per-core tables."""
    import ml_dtypes

    L = lens[t].astype(np.int64)                      # [N_EX]
    rank = np.argsort(-L, kind="stable")              # examples by length desc

    # slot s (0..1023) of core c takes example rank[s*8 + c]
    sel = rank.reshape(N_CORES * N_TILES * P // N_CORES, N_CORES)  # [1024, 8]
    # per-tile common max length (rank band head)
    lmax = tuple(int(L[rank[k * (N_CORES * P)]]) for k in range(N_TILES))
    ltot = int(sum(lmax))

    W16 = W.astype(ml_dtypes.bfloat16)                # shared across cores
    if FUSED_FP8:
        W8 = (W * W8_SCALE).astype(ml_dtypes.float8_e4m3)
        # dots-columns computed from the fp8 table need the scale divided out
        fp8_col = np.zeros(N_TILES * MAX_LEN, dtype=bool)
        for k, c0, c, na, nf in _plan(lmax):
            fp8_col[k * MAX_LEN + c0 + na : k * MAX_LEN + c0 + c] = True

    in_maps = []
    for c in range(N_CORES):
        ex = sel[:, c]                                # [1024] example ids
        xs_c = np.ascontiguousarray(x[ex]).astype(ml_dtypes.bfloat16)
        t_c = t[ex]
        node_c = paths[t_c]                           # [1024, MAX_LEN] int32
        code_c = codes[t_c]                           # [1024, MAX_LEN] f32
        L_c = L[ex]                                   # [1024]

        gidx_c = np.zeros((P, ltot), dtype=np.int32)
        nsc_c = np.zeros((P, N_TILES * MAX_LEN), dtype=np.float32)
        mbs_c = np.full((P, N_TILES * MAX_LEN), MASK_BIAS, dtype=np.float32)
        it0 = 0
        for k in range(N_TILES):
            lm = lmax[k]
            rows = slice(k * P, (k + 1) * P)
            valid = np.arange(lm)[None, :] < L_c[rows][:, None]   # [P, lm]
            gidx_c[:, it0 : it0 + lm] = np.where(valid, node_c[rows, :lm], 0)
            nsc_c[:, k * MAX_LEN : k * MAX_LEN + lm] = np.where(
                valid, -code_c[rows, :lm], 0.0
            )
            mbs_c[:, k * MAX_LEN : k * MAX_LEN + lm] = np.where(valid, 0.0, MASK_BIAS)
            it0 += lm

        im = {
            "xs": xs_c,
            "W16": W16,
            "gidx": gidx_c,
            "nsc": nsc_c,
            "mbs": mbs_c,
        }
        if FUSED_FP8:
            nsc_c[:, fp8_col] /= W8_SCALE
            im["W8"] = W8
        in_maps.append(im)
    return lmax, in_maps


def kernel(x, W, t, paths, codes, lens):
    from concourse import bass_utils

    lmax, in_maps = _prepare(
        np.asarray(x), np.asarray(W), np.asarray(t),
        np.asarray(paths), np.asarray(codes), np.asarray(lens),
    )
    nc = _prog_cache.get(lmax)
    if nc is None:
        nc = _build_program(lmax)
        _prog_cache[lmax] = nc

    res = bass_utils.run_bass_kernel_spmd(nc, in_maps, core_ids=list(range(N_CORES)))
    total = sum(r["out"].astype(np.float64).sum() for r in res.results)
    return np.float32(total)


# revision 39
# speedup vs baseline: 1.0133x; 1.0133x over previous
"""Hierarchical-softmax loss kernel for Trainium2 (8 NeuronCores).

Strategy
--------
Data-parallel over the n_ex dimension. Examples are globally sorted by
path length (descending) and dealt round-robin to the 8 cores so every
core sees a near-identical length profile. Within a core, examples are
tiled into 8 partition-tiles of 128; each tile's step loop runs only to
that tile's max length (sum ~114 of the dense 192 steps).

W is cast to bf16 on the host, halving the dominant gather traffic
(~29 MB/core HBM reads, ~81 us at the measured ~360 GB/s). Gathers are
batched 8 steps (1024 rows) per indirect DMA: SWDGE descriptor
generation costs ~1 us fixed per instruction but only 0.34 ns per
descriptor, so batching cuts the GpSimd engine load ~7x vs per-step
gathers (134 us -> ~30 us).

Per chunk, one batched DVE multiply (bf16 2x mode, ~440-520 ns/step)
produces all products; the per-step dot reductions are then split
11/16 to ScalarE copy-accumulate (~1.28 us/step) and 5/16 to a single
grouped DVE tensor_reduce (~0.96 us/step) so both engines finish
together (~97 us each, the kernel's wall). Fused mult+reduce ops
(tensor_tensor_reduce, custom-DVE affine_mul_reduce) do NOT compile on
the pinned walrus ("ISA wrong length"), and GpSimd tensor compute
inflates DVE/ACT times 25-30% via SBUF port contention, so neither is
used. First chunks are tapered (2/3/5 steps) and x tiles load lazily
to shorten the pipeline ramp; the last chunks lean on DVE because
ScalarE is the post-DMA straggler. The final softplus tail runs once
over all dots; the scalar sum over 8x128 partials happens on host.
"""

import os
import sys

import numpy as np

for _p in ("/opt/trn_rl_repo", "/root/.axon_site/_ro/trn_rl_repo"):
    if os.path.isdir(_p) and _p not in sys.path:
        sys.path.append(_p)

V = 50257
N_DEC = V - 1
D = 1024
N_EX = 8192
MAX_LEN = 24
N_CORES = 8
P = 128
N_TILES = N_EX // (N_CORES * P)  # 8 example-tiles of 128 per core
MASK_BIAS = -30.0                # softplus(-30) ~ 9e-14 == masked-out step

CHUNK = 8   # steps per gather batch
NA_NUM, NA_DEN = 5, 8   # fraction of steps reduced on ScalarE (rest on DVE)
# GpSimd compute is disabled: Q7 SBUF traffic contends with DVE 2x-port mode
# and inflates DVE/ACT instruction times ~25% (measured).
NP_NUM, NP_DEN = 0, 16   # fraction of steps whose multiply runs on GpSimd
PF = 3      # chunks of gather prefetch ahead of compute
FUSED_FP8 = False   # (fused DVE ops don't compile on the pinned walrus)
W8_SCALE = 32.0     # host-side W scale so e4m3 stays in normal range

_prog_cache: dict = {}


def _plan(lmax):
    """Chunk schedule shared by host prep and program builder.

    Returns [(k, c0, c, na, nf)]: tile k, step range [c0, c0+c), na steps
    ScalarE-reduced (first na of the chunk), nf = c - na fused on DVE.
    """
    chunks = []
    acc = 0  # Bresenham accumulator so the ScalarE fraction holds globally
    for k, lm in enumerate(lmax):
        lm = int(lm)
        if k == 0 and lm > CHUNK:
            # taper the first chunks so the pipeline fills quickly
            sizes = [2, 3, 5]
            while sum(sizes) + CHUNK <= lm:
                sizes.append(CHUNK)
            rest = lm - sum(sizes)
            if rest > 0:
                sizes.append(rest)
        else:
            sizes = [CHUNK] * (lm // CHUNK)
            if lm % CHUNK:
                sizes.append(lm % CHUNK)
        c0 = 0
        for c in sizes:
            acc += c * NA_NUM
            na = acc // NA_DEN
            acc -= na * NA_DEN
            chunks.append([k, c0, c, na, c - na])
            c0 += c
    # tail chunks lean on DVE: ScalarE accums are the post-DMA straggler
    for ch in chunks[-2:]:
        ch[3] = ch[2] // 3
        ch[4] = ch[2] - ch[3]
    return [tuple(ch) for ch in chunks]


def _patch_tail_drain(tile, mybir, bass_rust):
    """The pinned walrus encodes only a limited number of sync-waits per CTRL
    instruction, but Tile's kernel-tail Drain carries one wait per active
    processor lane. Spread the extra waits over single-wait NOPs."""
    if getattr(tile.TileContext._drain_and_barrier, "_split_waits", False):
        return

    def _drain_and_barrier(self, tick_clock, wait_clock):
        nc = self.nc
        drain_inst = nc.sync.drain()
        wait_clock.add_sem_waits(
            drain_inst.ins, bass_rust.ScopedClock({None: tick_clock.global_clock})
        )
        si = drain_inst.ins.sync_info
        waits = list(si.on_wait or [])
        if len(waits) > 1:
            si.on_wait = waits[:1]
            for w in waits[1:]:
                nop = nc.sync.nop(nofuse=True)
                nop.ins.sync_info = mybir.SyncInfo(on_wait=[w], on_update=[])
        nc.all_engine_barrier()
        popped = nc._tile_sem_poison_stack.pop()
        assert popped is self._sem_poison
        nc.clear_and_free_semaphores(list(self.sems.allocated().values()))
        nc.all_engine_barrier()

    _drain_and_barrier._split_waits = True
    tile.TileContext._drain_and_barrier = _drain_and_barrier


def _split_multiwait_instructions(nc, mybir, maxw=1):
    """Hoist extra sem-waits from any instruction onto single-wait NOPs placed
    immediately before it on the same engine (same aggregate wait semantics)."""
    f = nc.m.functions[0]
    tail = nc.cur_bb.bb
    blocks = list(f.blocks)
    if not any(b.name == tail.name for b in blocks):
        blocks.append(tail)
    for blk in blocks:
        snapshot = list(blk.instructions)
        heavy = [
            i for i in snapshot
            if i.sync_info and i.sync_info.on_wait and len(i.sync_info.on_wait) > maxw
        ]
        if not heavy:
            continue
        pre_len = len(tail.instructions)
        n_created = 0
        new_list = []
        for inst in snapshot:
            si = inst.sync_info
            if si and si.on_wait and len(si.on_wait) > maxw:
                waits = list(si.on_wait)
                extra, keep = waits[:-maxw], waits[-maxw:]
                si.on_wait = keep
                for w in extra:
                    nop = nc.engines[inst.engine].nop(nofuse=True)
                    nop.ins.sync_info = mybir.SyncInfo(on_wait=[w], on_update=[])
                    new_list.append(nop.ins)
                    n_created += 1
            new_list.append(inst)
        # builder appended the fresh NOPs to the tail block; strip them there
        t = list(tail.instructions)
        assert len(t) == pre_len + n_created
        if blk.name == tail.name:
            blk.instructions = new_list
        else:
            tail.instructions = t[:pre_len]
            blk.instructions = new_list


def _build_program(lmax: tuple):
    from concourse import bass, mybir
    import concourse.tile as tile
    import bass_rust

    _patch_tail_drain(tile, mybir, bass_rust)
    chunks = _plan(lmax)
    ltot = int(sum(lmax))

    nc = bass.Bass("TRN2", target_bir_lowering=False,
                   dynamic_dma_scratch_size=2 ** 16)
    f32 = mybir.dt.float32
    bf16 = mybir.dt.bfloat16

    xs = nc.declare_dram_parameter("xs", [N_TILES * P, D], bf16, isOutput=False)
    W16 = nc.declare_dram_parameter("W16", [N_DEC, D], bf16, isOutput=False)
    if FUSED_FP8:
        W8 = nc.declare_dram_parameter("W8", [N_DEC, D], mybir.dt.float8e4,
                                       isOutput=False)
    gidx = nc.declare_dram_parameter("gidx", [P, ltot], mybir.dt.int32, isOutput=False)
    nsc = nc.declare_dram_parameter("nsc", [P, N_TILES * MAX_LEN], f32, isOutput=False)
    mbs = nc.declare_dram_parameter("mbs", [P, N_TILES * MAX_LEN], f32, isOutput=False)
    out = nc.declare_dram_parameter("out", [P, 1], f32, isOutput=True)

    it0 = np.concatenate([[0], np.cumsum(lmax)]).astype(int)  # gidx col base per tile

    with tile.TileContext(nc) as tc:
        with (
            tc.tile_pool(name="xpool", bufs=1) as xpool,
            tc.tile_pool(name="gpool", bufs=PF + 1) as gpool,
            tc.tile_pool(name="meta", bufs=1) as meta,
            tc.tile_pool(name="pspool", bufs=3) as pspool,
            tc.tile_pool(name="dpool", bufs=2) as dpool,
            tc.tile_pool(name="outp", bufs=1) as outp,
        ):
            gidx_t = meta.tile([P, ltot], mybir.dt.int32, tag="gidx")
            nsc_t = meta.tile([P, N_TILES * MAX_LEN], f32, tag="nsc")
            mbs_t = meta.tile([P, N_TILES * MAX_LEN], f32, tag="mbs")
            nc.sync.dma_start(out=gidx_t[:], in_=gidx[:, :])

            parts = outp.tile([P, 1], f32, tag="parts")
            pa = outp.tile([P, 1], f32, tag="pa")
            pb = outp.tile([P, 1], f32, tag="pb")

            # x tiles load lazily (tile k's load is emitted one tile ahead of
            # first use) so the startup DMA window belongs to the gathers
            xt = []
            for k in range(N_TILES):
                xt.append(xpool.tile([P, D], bf16, tag=f"x{k}", name=f"x{k}"))
            x_loaded = [False] * N_TILES

            def load_x(k):
                if k < N_TILES and not x_loaded[k]:
                    nc.sync.dma_start(out=xt[k][:], in_=xs[k * P : (k + 1) * P, :])
                    x_loaded[k] = True

            load_x(0)
            load_x(1)

            # one dots buffer for all tiles; padded columns stay 0 via memset
            dots = dpool.tile([P, N_TILES * MAX_LEN], f32, tag="dots")
            nc.vector.memset(dots[:], 0.0)

            g_tiles = {}

            def emit_gather(i):
                k, c0, c, na, nd = chunks[i]
                g = gpool.tile([P, CHUNK * D], bf16, tag="g")
                nc.gpsimd.indirect_dma_start(
                    out=g[:, : c * D],
                    out_offset=None,
                    in_=W16[:, :],
                    in_offset=bass.IndirectOffsetOnAxis(
                        ap=gidx_t[:, it0[k] + c0 : it0[k] + c0 + c], axis=0
                    ),
                )
                g_tiles[i] = g

            for i in range(min(PF + 1, len(chunks))):
                emit_gather(i)

            for i, (k, c0, c, na, nd) in enumerate(chunks):
                g = g_tiles.pop(i)
                dbase = k * MAX_LEN + c0
                load_x(k + 1)
                if i == len(chunks) - 6:
                    # gather stream is winding down: fetch the tail tables
                    nc.sync.dma_start(out=nsc_t[:], in_=nsc[:, :])
                    nc.sync.dma_start(out=mbs_t[:], in_=mbs[:, :])
                ps = pspool.tile([P, CHUNK * D], bf16, tag="ps")
                x3 = xt[k][:].unsqueeze(1).broadcast_to([P, c, D])
                nc.vector.tensor_tensor(
                    out=ps[:, : c * D].rearrange("p (n d) -> p n d", d=D),
                    in0=x3,
                    in1=g[:, : c * D].rearrange("p (n d) -> p n d", d=D),
                    op=mybir.AluOpType.mult,
                )
                dump = pspool.tile([P, D], bf16, tag="dump", bufs=2)
                for j in range(na):
                    nc.scalar.activation(
                        out=dump[:], in_=ps[:, j * D : (j + 1) * D],
                        func=mybir.ActivationFunctionType.Copy,
                        accum_out=dots[:, dbase + j : dbase + j + 1],
                    )
                if nd > 0:
                    # 2x half-add over the DVE block halves the 1x reduce read
                    H = D // 2
                    ps3 = ps[:, na * D : c * D].rearrange("p (n d) -> p n d", d=D)
                    ph = pspool.tile([P, CHUNK * H], bf16, tag="ph", bufs=1)
                    nc.vector.tensor_tensor(
                        out=ph[:, : nd * H].rearrange("p (n h) -> p n h", h=H),
                        in0=ps3[:, :, :H],
                        in1=ps3[:, :, H:],
                        op=mybir.AluOpType.add,
                    )
                    nc.vector.tensor_reduce(
                        out=dots[:, dbase + na : dbase + c],
                        in_=ph[:, : nd * H].rearrange(
                            "p (n h) -> p n h", h=H
                        ),
                        axis=mybir.AxisListType.X,
                        op=mybir.AluOpType.add,
                    )
                if i + PF + 1 < len(chunks):
                    emit_gather(i + PF + 1)

            # v = dot * (-code) + mask_bias over all tiles at once
            nc.vector.tensor_tensor(
                out=dots[:], in0=dots[:], in1=nsc_t[:], op=mybir.AluOpType.mult
            )
            nc.vector.tensor_tensor(
                out=dots[:], in0=dots[:], in1=mbs_t[:], op=mybir.AluOpType.add
            )
            # stable softplus(v) = relu(v) + ln(1 + exp(-|v|));
            # abs/exp/ln/relu share one ACT table set (natural_log_exp).
            va = dpool.tile([P, N_TILES * MAX_LEN], f32, tag="va")
            nc.scalar.activation(
                out=va[:], in_=dots[:], func=mybir.ActivationFunctionType.Abs
            )
            ve = dpool.tile([P, N_TILES * MAX_LEN], f32, tag="ve")
            nc.scalar.activation(
                out=ve[:], in_=va[:],
                func=mybir.ActivationFunctionType.Exp, scale=-1.0,
            )
            vl = dpool.tile([P, N_TILES * MAX_LEN], f32, tag="vl")
            nc.scalar.activation(
                out=vl[:], in_=ve[:],
                func=mybir.ActivationFunctionType.Ln, bias=1.0,
                accum_out=pb[:, :],
            )
            vr = dpool.tile([P, N_TILES * MAX_LEN], f32, tag="vr")
            nc.scalar.activation(
                out=vr[:], in_=dots[:],
                func=mybir.ActivationFunctionType.Relu,
                accum_out=pa[:, :],
            )

            nc.vector.tensor_tensor(
                out=parts[:], in0=pa[:], in1=pb[:], op=mybir.AluOpType.add
            )
            nc.sync.dma_start(out=out[:, :], in_=parts[:])

    _split_multiwait_instructions(nc, mybir)
    return nc


def _prepare(x, W, t, paths, codes, lens):
    """Host-side index prep: length-sorted round-robin shard + per-core tables."""
    import ml_dtypes

    L = lens[t].astype(np.int64)                      # [N_EX]
    rank = np.argsort(-L, kind="stable")              # examples by length desc

    # slot s (0..1023) of core c takes example rank[s*8 + c]
    sel = rank.reshape(N_CORES * N_TILES * P // N_CORES, N_CORES)  # [1024, 8]
    # per-tile common max length (rank band head)
    lmax = tuple(int(L[rank[k * (N_CORES * P)]]) for k in range(N_TILES))
    ltot = int(sum(lmax))

    W16 = W.astype(ml_dtypes.bfloat16)                # shared across cores
    if FUSED_FP8:
        W8 = (W * W8_SCALE).astype(ml_dtypes.float8_e4m3)
        # dots-columns computed from the fp8 table need the scale divided out
        fp8_col = np.zeros(N_TILES * MAX_LEN, dtype=bool)
        for k, c0, c, na, nf in _plan(lmax):
            fp8_col[k * MAX_LEN + c0 + na : k * MAX_LEN + c0 + c] = True

    in_maps = []
    for c in range(N_CORES):
        ex = sel[:, c]                                # [1024] example ids
        xs_c = np.ascontiguousarray(x[ex]).astype(ml_dtypes.bfloat16)
        t_c = t[ex]
        node_c = paths[t_c]                           # [1024, MAX_LEN] int32
        code_c = codes[t_c]                           # [1024, MAX_LEN] f32
        L_c = L[ex]                                   # [1024]

        gidx_c = np.zeros((P, ltot), dtype=np.int32)
        nsc_c = np.zeros((P, N_TILES * MAX_LEN), dtype=np.float32)
        mbs_c = np.full((P, N_TILES * MAX_LEN), MASK_BIAS, dtype=np.float32)
        it0 = 0
        for k in range(N_TILES):
            lm = lmax[k]
            rows = slice(k * P, (k + 1) * P)
            valid = np.arange(lm)[None, :] < L_c[rows][:, None]   # [P, lm]
            gidx_c[:, it0 : it0 + lm] = np.where(valid, node_c[rows, :lm], 0)
            nsc_c[:, k * MAX_LEN : k * MAX_LEN + lm] = np.where(
                valid, -code_c[rows, :lm], 0.0
            )
            mbs_c[:, k * MAX_LEN : k * MAX_LEN + lm] = np.where(valid, 0.0, MASK_BIAS)
            it0 += lm

        im = {
            "xs": xs_c,
            "W16": W16,
            "gidx": gidx_c,
            "nsc": nsc_c,
            "mbs": mbs_c,
        }
        if FUSED_FP8:
            nsc_c[:, fp8_col] /= W8_SCALE
            im["W8"] = W8
        in_maps.append(im)
    return lmax, in_maps


def kernel(x, W, t, paths, codes, lens):
    from concourse import bass_utils

    lmax, in_maps = _prepare(
        np.asarray(x), np.asarray(W), np.asarray(t),
        np.asarray(paths), np.asarray(codes), np.asarray(lens),
    )
    nc = _prog_cache.get(lmax)
    if nc is None:
        nc = _build_program(lmax)
        _prog_cache[lmax] = nc

    res = bass_utils.run_bass_kernel_spmd(nc, in_maps, core_ids=list(range(N_CORES)))
    total = sum(r["out"].astype(np.float64).sum() for r in res.results)
    return np.float32(total)


# revision 41
# speedup vs baseline: 1.1451x; 1.1301x over previous
"""Hierarchical-softmax loss kernel for Trainium2 (8 NeuronCores).

Strategy
--------
Data-parallel over the n_ex dimension. Examples are globally sorted by
path length (descending) and dealt round-robin to the 8 cores so every
core sees a near-identical length profile. Within a core, examples are
tiled into 8 partition-tiles of 128; each tile's step loop runs only to
that tile's max length (sum ~114 of the dense 192 steps).

W is cast to bf16 on the host, halving the dominant gather traffic
(~29 MB/core HBM reads, ~81 us at the measured ~360 GB/s). Gathers are
batched 8 steps (1024 rows) per indirect DMA: SWDGE descriptor
generation costs ~1 us fixed per instruction but only 0.34 ns per
descriptor, so batching cuts the GpSimd engine load ~7x vs per-step
gathers (134 us -> ~30 us).

Per chunk, one batched DVE multiply (bf16 2x mode, ~440-520 ns/step)
produces all products; the per-step dot reductions are then split
11/16 to ScalarE copy-accumulate (~1.28 us/step) and 5/16 to a single
grouped DVE tensor_reduce (~0.96 us/step) so both engines finish
together (~97 us each, the kernel's wall). Fused mult+reduce ops
(tensor_tensor_reduce, custom-DVE affine_mul_reduce) do NOT compile on
the pinned walrus ("ISA wrong length"), and GpSimd tensor compute
inflates DVE/ACT times 25-30% via SBUF port contention, so neither is
used. First chunks are tapered (2/3/5 steps) and x tiles load lazily
to shorten the pipeline ramp; the last chunks lean on DVE because
ScalarE is the post-DMA straggler. The final softplus tail runs once
over all dots; the scalar sum over 8x128 partials happens on host.
"""

import os
import sys

import numpy as np

for _p in ("/opt/trn_rl_repo", "/root/.axon_site/_ro/trn_rl_repo"):
    if os.path.isdir(_p) and _p not in sys.path:
        sys.path.append(_p)

V = 50257
N_DEC = V - 1
D = 1024
N_EX = 8192
MAX_LEN = 24
N_CORES = 8
P = 128
N_TILES = N_EX // (N_CORES * P)  # 8 example-tiles of 128 per core
MASK_BIAS = -30.0                # softplus(-30) ~ 9e-14 == masked-out step

CHUNK = 8   # steps per gather batch
NA_NUM, NA_DEN = 11, 16  # fraction of steps reduced on ScalarE (rest on DVE)
# GpSimd compute is disabled: Q7 SBUF traffic contends with DVE 2x-port mode
# and inflates DVE/ACT instruction times ~25% (measured).
NP_NUM, NP_DEN = 0, 16   # fraction of steps whose multiply runs on GpSimd
PF = 3      # chunks of gather prefetch ahead of compute
FUSED_FP8 = False   # (fused DVE ops don't compile on the pinned walrus)
W8_SCALE = 32.0     # host-side W scale so e4m3 stays in normal range

_prog_cache: dict = {}


def _plan(lmax):
    """Chunk schedule shared by host prep and program builder.

    Returns [(k, c0, c, na, nf)]: tile k, step range [c0, c0+c), na steps
    ScalarE-reduced (first na of the chunk), nf = c - na fused on DVE.
    """
    chunks = []
    acc = 0  # Bresenham accumulator so the ScalarE fraction holds globally
    for k, lm in enumerate(lmax):
        lm = int(lm)
        if k == 0 and lm > CHUNK:
            # taper the first chunks so the pipeline fills quickly
            sizes = [2, 3, 5]
            while sum(sizes) + CHUNK <= lm:
                sizes.append(CHUNK)
            rest = lm - sum(sizes)
            if rest > 0:
                sizes.append(rest)
        else:
            sizes = [CHUNK] * (lm // CHUNK)
            if lm % CHUNK:
                sizes.append(lm % CHUNK)
        c0 = 0
        for c in sizes:
            acc += c * NA_NUM
            na = acc // NA_DEN
            acc -= na * NA_DEN
            chunks.append([k, c0, c, na, c - na])
            c0 += c
    # tail chunks lean on DVE: ScalarE accums are the post-DMA straggler
    for ch in chunks[-2:]:
        ch[3] = ch[2] // 3
        ch[4] = ch[2] - ch[3]
    return [tuple(ch) for ch in chunks]


def _patch_tail_drain(tile, mybir, bass_rust):
    """The pinned walrus encodes only a limited number of sync-waits per CTRL
    instruction, but Tile's kernel-tail Drain carries one wait per active
    processor lane. Spread the extra waits over single-wait NOPs."""
    if getattr(tile.TileContext._drain_and_barrier, "_split_waits", False):
        return

    def _drain_and_barrier(self, tick_clock, wait_clock):
        nc = self.nc
        drain_inst = nc.sync.drain()
        wait_clock.add_sem_waits(
            drain_inst.ins, bass_rust.ScopedClock({None: tick_clock.global_clock})
        )
        si = drain_inst.ins.sync_info
        waits = list(si.on_wait or [])
        if len(waits) > 1:
            si.on_wait = waits[:1]
            for w in waits[1:]:
                nop = nc.sync.nop(nofuse=True)
                nop.ins.sync_info = mybir.SyncInfo(on_wait=[w], on_update=[])
        nc.all_engine_barrier()
        popped = nc._tile_sem_poison_stack.pop()
        assert popped is self._sem_poison
        nc.clear_and_free_semaphores(list(self.sems.allocated().values()))
        nc.all_engine_barrier()

    _drain_and_barrier._split_waits = True
    tile.TileContext._drain_and_barrier = _drain_and_barrier


def _split_multiwait_instructions(nc, mybir, maxw=1):
    """Hoist extra sem-waits from any instruction onto single-wait NOPs placed
    immediately before it on the same engine (same aggregate wait semantics)."""
    f = nc.m.functions[0]
    tail = nc.cur_bb.bb
    blocks = list(f.blocks)
    if not any(b.name == tail.name for b in blocks):
        blocks.append(tail)
    for blk in blocks:
        snapshot = list(blk.instructions)
        heavy = [
            i for i in snapshot
            if i.sync_info and i.sync_info.on_wait and len(i.sync_info.on_wait) > maxw
        ]
        if not heavy:
            continue
        pre_len = len(tail.instructions)
        n_created = 0
        new_list = []
        for inst in snapshot:
            si = inst.sync_info
            if si and si.on_wait and len(si.on_wait) > maxw:
                waits = list(si.on_wait)
                extra, keep = waits[:-maxw], waits[-maxw:]
                si.on_wait = keep
                for w in extra:
                    nop = nc.engines[inst.engine].nop(nofuse=True)
                    nop.ins.sync_info = mybir.SyncInfo(on_wait=[w], on_update=[])
                    new_list.append(nop.ins)
                    n_created += 1
            new_list.append(inst)
        # builder appended the fresh NOPs to the tail block; strip them there
        t = list(tail.instructions)
        assert len(t) == pre_len + n_created
        if blk.name == tail.name:
            blk.instructions = new_list
        else:
            tail.instructions = t[:pre_len]
            blk.instructions = new_list


def _build_program(lmax: tuple):
    from concourse import bass, mybir
    import concourse.tile as tile
    import bass_rust

    _patch_tail_drain(tile, mybir, bass_rust)
    chunks = _plan(lmax)
    ltot = int(sum(lmax))

    nc = bass.Bass("TRN2", target_bir_lowering=False,
                   dynamic_dma_scratch_size=2 ** 16)
    f32 = mybir.dt.float32
    bf16 = mybir.dt.bfloat16

    xs = nc.declare_dram_parameter("xs", [N_TILES * P, D], bf16, isOutput=False)
    W16 = nc.declare_dram_parameter("W16", [N_DEC, D], bf16, isOutput=False)
    if FUSED_FP8:
        W8 = nc.declare_dram_parameter("W8", [N_DEC, D], mybir.dt.float8e4,
                                       isOutput=False)
    gidx = nc.declare_dram_parameter("gidx", [P, ltot], mybir.dt.int32, isOutput=False)
    nsc = nc.declare_dram_parameter("nsc", [P, N_TILES * MAX_LEN], f32, isOutput=False)
    mbs = nc.declare_dram_parameter("mbs", [P, N_TILES * MAX_LEN], f32, isOutput=False)
    out = nc.declare_dram_parameter("out", [P, 1], f32, isOutput=True)

    it0 = np.concatenate([[0], np.cumsum(lmax)]).astype(int)  # gidx col base per tile

    with tile.TileContext(nc) as tc:
        with (
            tc.tile_pool(name="xpool", bufs=1) as xpool,
            tc.tile_pool(name="gpool", bufs=PF + 1) as gpool,
            tc.tile_pool(name="meta", bufs=1) as meta,
            tc.tile_pool(name="pspool", bufs=3) as pspool,
            tc.tile_pool(name="dpool", bufs=2) as dpool,
            tc.tile_pool(name="outp", bufs=1) as outp,
        ):
            gidx_t = meta.tile([P, ltot], mybir.dt.int32, tag="gidx")
            nsc_t = meta.tile([P, N_TILES * MAX_LEN], f32, tag="nsc")
            mbs_t = meta.tile([P, N_TILES * MAX_LEN], f32, tag="mbs")
            nc.sync.dma_start(out=gidx_t[:], in_=gidx[:, :])

            parts = outp.tile([P, 1], f32, tag="parts")
            pa = outp.tile([P, 1], f32, tag="pa")
            pb = outp.tile([P, 1], f32, tag="pb")

            # x tiles load lazily (tile k's load is emitted one tile ahead of
            # first use) so the startup DMA window belongs to the gathers
            xt = []
            for k in range(N_TILES):
                xt.append(xpool.tile([P, D], bf16, tag=f"x{k}", name=f"x{k}"))
            x_loaded = [False] * N_TILES

            def load_x(k):
                if k < N_TILES and not x_loaded[k]:
                    nc.sync.dma_start(out=xt[k][:], in_=xs[k * P : (k + 1) * P, :])
                    x_loaded[k] = True

            load_x(0)
            load_x(1)

            # one dots buffer for all tiles; padded columns stay 0 via memset
            dots = dpool.tile([P, N_TILES * MAX_LEN], f32, tag="dots")
            nc.vector.memset(dots[:], 0.0)

            g_tiles = {}

            def emit_gather(i):
                k, c0, c, na, nd = chunks[i]
                g = gpool.tile([P, CHUNK * D], bf16, tag="g")
                nc.gpsimd.indirect_dma_start(
                    out=g[:, : c * D],
                    out_offset=None,
                    in_=W16[:, :],
                    in_offset=bass.IndirectOffsetOnAxis(
                        ap=gidx_t[:, it0[k] + c0 : it0[k] + c0 + c], axis=0
                    ),
                )
                g_tiles[i] = g

            for i in range(min(PF + 1, len(chunks))):
                emit_gather(i)

            for i, (k, c0, c, na, nd) in enumerate(chunks):
                g = g_tiles.pop(i)
                dbase = k * MAX_LEN + c0
                load_x(k + 1)
                if i == len(chunks) - 6:
                    # gather stream is winding down: fetch the tail tables
                    nc.sync.dma_start(out=nsc_t[:], in_=nsc[:, :])
                    nc.sync.dma_start(out=mbs_t[:], in_=mbs[:, :])
                ps = pspool.tile([P, CHUNK * D], bf16, tag="ps")
                x3 = xt[k][:].unsqueeze(1).broadcast_to([P, c, D])
                nc.vector.tensor_tensor(
                    out=ps[:, : c * D].rearrange("p (n d) -> p n d", d=D),
                    in0=x3,
                    in1=g[:, : c * D].rearrange("p (n d) -> p n d", d=D),
                    op=mybir.AluOpType.mult,
                )
                dump = pspool.tile([P, D], bf16, tag="dump")
                for j in range(na):
                    nc.scalar.activation(
                        out=dump[:], in_=ps[:, j * D : (j + 1) * D],
                        func=mybir.ActivationFunctionType.Copy,
                        accum_out=dots[:, dbase + j : dbase + j + 1],
                    )
                if nd > 0:
                    nc.vector.tensor_reduce(
                        out=dots[:, dbase + na : dbase + c],
                        in_=ps[:, na * D : c * D].rearrange(
                            "p (n d) -> p n d", d=D
                        ),
                        axis=mybir.AxisListType.X,
                        op=mybir.AluOpType.add,
                    )
                if i + PF + 1 < len(chunks):
                    emit_gather(i + PF + 1)

            # v = dot * (-code) + mask_bias over all tiles at once
            nc.vector.tensor_tensor(
                out=dots[:], in0=dots[:], in1=nsc_t[:], op=mybir.AluOpType.mult
            )
            nc.vector.tensor_tensor(
                out=dots[:], in0=dots[:], in1=mbs_t[:], op=mybir.AluOpType.add
            )
            # stable softplus(v) = relu(v) + ln(1 + exp(-|v|));
            # abs/exp/ln/relu share one ACT table set (natural_log_exp).
            va = dpool.tile([P, N_TILES * MAX_LEN], f32, tag="va")
            nc.scalar.activation(
                out=va[:], in_=dots[:], func=mybir.ActivationFunctionType.Abs
            )
            ve = dpool.tile([P, N_TILES * MAX_LEN], f32, tag="ve")
            nc.scalar.activation(
                out=ve[:], in_=va[:],
                func=mybir.ActivationFunctionType.Exp, scale=-1.0,
            )
            vl = dpool.tile([P, N_TILES * MAX_LEN], f32, tag="vl")
            nc.scalar.activation(
                out=vl[:], in_=ve[:],
                func=mybir.ActivationFunctionType.Ln, bias=1.0,
                accum_out=pb[:, :],
            )
            vr = dpool.tile([P, N_TILES * MAX_LEN], f32, tag="vr")
            nc.scalar.activation(
                out=vr[:], in_=dots[:],
                func=mybir.ActivationFunctionType.Relu,
                accum_out=pa[:, :],
            )

            nc.vector.tensor_tensor(
                out=parts[:], in0=pa[:], in1=pb[:], op=mybir.AluOpType.add
            )
            nc.sync.dma_start(out=out[:, :], in_=parts[:])

    _split_multiwait_instructions(nc, mybir)
    return nc


def _prepare(x, W, t, paths, codes, lens):
    """Host-side index prep: length-sorted round-robin shard + per-core tables."""
    import ml_dtypes

    L = lens[t].astype(np.int64)                      # [N_EX]
    rank = np.argsort(-L, kind="stable")              # examples by length desc

    # slot s (0..1023) of core c takes example rank[s*8 + c]
    sel = rank.reshape(N_CORES * N_TILES * P // N_CORES, N_CORES)  # [1024, 8]
    # per-tile common max length (rank band head)
    lmax = tuple(int(L[rank[k * (N_CORES * P)]]) for k in range(N_TILES))
    ltot = int(sum(lmax))

    W16 = W.astype(ml_dtypes.bfloat16)                # shared across cores
    if FUSED_FP8:
        W8 = (W * W8_SCALE).astype(ml_dtypes.float8_e4m3)
        # dots-columns computed from the fp8 table need the scale divided out
        fp8_col = np.zeros(N_TILES * MAX_LEN, dtype=bool)
        for k, c0, c, na, nf in _plan(lmax):
            fp8_col[k * MAX_LEN + c0 + na : k * MAX_LEN + c0 + c] = True

    in_maps = []
    for c in range(N_CORES):
        ex = sel[:, c]                                # [1024] example ids
        xs_c = np.ascontiguousarray(x[ex]).astype(ml_dtypes.bfloat16)
        t_c = t[ex]
        node_c = paths[t_c]                           # [1024, MAX_LEN] int32
        code_c = codes[t_c]                           # [1024, MAX_LEN] f32
        L_c = L[ex]                                   # [1024]

        gidx_c = np.zeros((P, ltot), dtype=np.int32)
        nsc_c = np.zeros((P, N_TILES * MAX_LEN), dtype=np.float32)
        mbs_c = np.full((P, N_TILES * MAX_LEN), MASK_BIAS, dtype=np.float32)
        it0 = 0
        for k in range(N_TILES):
            lm = lmax[k]
            rows = slice(k * P, (k + 1) * P)
            valid = np.arange(lm)[None, :] < L_c[rows][:, None]   # [P, lm]
            gidx_c[:, it0 : it0 + lm] = np.where(valid, node_c[rows, :lm], 0)
            nsc_c[:, k * MAX_LEN : k * MAX_LEN + lm] = np.where(
                valid, -code_c[rows, :lm], 0.0
            )
            mbs_c[:, k * MAX_LEN : k * MAX_LEN + lm] = np.where(valid, 0.0, MASK_BIAS)
            it0 += lm

        im = {
            "xs": xs_c,
            "W16": W16,
            "gidx": gidx_c,
            "nsc": nsc_c,
            "mbs": mbs_c,
        }
        if FUSED_FP8:
            nsc_c[:, fp8_col] /= W8_SCALE
            im["W8"] = W8
        in_maps.append(im)
    return lmax, in_maps


def kernel(x, W, t, paths, codes, lens):
    from concourse import bass_utils

    lmax, in_maps = _prepare(
        np.asarray(x), np.asarray(W), np.asarray(t),
        np.asarray(paths), np.asarray(codes), np.asarray(lens),
    )
    nc = _prog_cache.get(lmax)
    if nc is None:
        nc = _build_program(lmax)
        _prog_cache[lmax] = nc

    res = bass_utils.run_bass_kernel_spmd(nc, in_maps, core_ids=list(range(N_CORES)))
    total = sum(r["out"].astype(np.float64).sum() for r in res.results)
    return np.float32(total)
